# revision 13
# baseline (speedup 1.0000x reference)
"""Trainium2 Bass kernel for nn_DecoderAblationAttModule.

Strategy (8 NeuronCores, SPMD, no collectives):
  NEFF-A (recurrence): data-parallel over batch. Rows are assigned
    round-robin after the length sort (core k gets sorted rows k, k+8, ...),
    32 rows/core. Per step: additive attention (PE + DVE/ACT), softmax via
    an exp/matmul-reduction trick, attention-weighted feature sum via
    block-diagonal matmuls, LSTM gates as matmuls with the large input
    weight streamed from HBM in bf16, LSTM elementwise, and capture of
    h_t / aw_t (bf16) for the deferred output projection.
  NEFF-B (projection): vocab-parallel. Each core computes its 1500-column
    vocab slice of  pred = h @ Wh^T + aw @ Wi^T + b  for the active
    (row, step) prefix only (rows are length-sorted, so active rows at
    step t form a prefix of size count_t).
Host: stable length sort, embedding gather, weight/activation layout
shuffles, final scatter into the zero-initialized output.
"""

import numpy as np
import ml_dtypes

import concourse.bass as bass
import concourse.mybir as mybir
from concourse import bacc
from concourse import bass_utils
from concourse.tile import TileContext

BF16 = mybir.dt.bfloat16
F32 = mybir.dt.float32
NPBF = ml_dtypes.bfloat16

# Problem constants
B, N, FT, TSEQ = 256, 36, 2048, 20
A, D, E, V = 512, 1024, 512, 12000
T = TSEQ - 1            # 19 decode steps
NC = 8                  # cores
BL = B // NC            # 32 rows per core
VL = V // NC            # 1500 vocab cols per core
VLP = 1536              # padded to 12*128
KO_F, KO_E, KO_H = FT // 128, E // 128, D // 128   # 16, 4, 8
KO = KO_F + KO_E + KO_H                            # 28
NB = 1152               # N * BL (attention free dim)

TRACE = False
LAST_EXEC_NS = []       # exec_time_ns of the last kernel() call's NEFFs

_cache = {}


def _ident(n, dt):
    return np.eye(n, dtype=dt)


# ----------------------------------------------------------------------------
# NEFF-A: recurrence
# ----------------------------------------------------------------------------

def build_neff_a():
    NROW = T * BL           # 608 (t, j) rows for the emb-gates precompute
    nc = bacc.Bacc("TRN2")
    i_fnb = nc.dram_tensor("fnb", [128, N // 4, FT], BF16, kind="ExternalInput")
    i_ftr = nc.dram_tensor("ftr", [128, KO_F, NB], BF16, kind="ExternalInput")
    i_wfT = nc.dram_tensor("wfT", [128, KO_F, A], BF16, kind="ExternalInput")
    i_wdT = nc.dram_tensor("wdT", [128, KO_H, A], BF16, kind="ExternalInput")
    i_waT = nc.dram_tensor("waT", [128, A // 128, 1], BF16, kind="ExternalInput")
    i_iab = nc.dram_tensor("iab", [128, A // 128], F32, kind="ExternalInput")
    i_whh = nc.dram_tensor("whh", [128, KO_H, 4 * D], BF16, kind="ExternalInput")
    i_wie = nc.dram_tensor("wie", [128, KO_E, 4 * D], BF16, kind="ExternalInput")
    i_wstr = nc.dram_tensor("wstr", [KO_F, 128, 4 * D], BF16, kind="ExternalInput")
    i_emb = nc.dram_tensor("embA", [128, KO_E, NROW], BF16, kind="ExternalInput")
    i_i32b = nc.dram_tensor("i32b", [128, BL], BF16, kind="ExternalInput")
    i_i32f = nc.dram_tensor("i32f", [128, BL], F32, kind="ExternalInput")
    i_idb = nc.dram_tensor("idb", [32, 32], BF16, kind="ExternalInput")
    o_h = nc.dram_tensor("ho", [T, BL, D], BF16, kind="ExternalOutput")
    o_aw = nc.dram_tensor("awo", [T, BL, FT], BF16, kind="ExternalOutput")
    d_eg = nc.dram_tensor("eg", [NROW, 4 * D], BF16)     # internal scratch

    with TileContext(nc) as tc:
        with tc.tile_pool(name="const", bufs=1) as cpool:
            wdT = cpool.tile([128, KO_H, A], BF16)
            waT = cpool.tile([128, A // 128, 1], BF16)
            iab = cpool.tile([128, A // 128], F32)
            i32b = cpool.tile([128, BL], BF16)
            i32f = cpool.tile([128, BL], F32)
            idb = cpool.tile([32, 32], BF16)
            fnb = cpool.tile([128, N // 4, FT], BF16)
            nc.sync.dma_start(out=wdT[:], in_=i_wdT[:])
            nc.sync.dma_start(out=waT[:], in_=i_waT[:])
            nc.sync.dma_start(out=iab[:], in_=i_iab[:])
            nc.sync.dma_start(out=i32b[:], in_=i_i32b[:])
            nc.sync.dma_start(out=i32f[:], in_=i_i32f[:])
            nc.sync.dma_start(out=idb[:], in_=i_idb[:])
            nc.sync.dma_start(out=fnb[:], in_=i_fnb[:])

            with tc.tile_pool(name="img", bufs=1) as ipool:
                # imgT[p_a, ao, n*32+j] = (Wf @ feats_j_n)[a] + bf[a] + bd[a]
                imgT = ipool.tile([128, A // 128, NB], BF16)

                # --- precompute img_att + emb-gates (transient tensors) ---
                with tc.tile_pool(name="pre", bufs=1) as prepool, \
                     tc.tile_pool(name="preo", bufs=4) as preo, \
                     tc.tile_pool(name="preps", bufs=4, space="PSUM") as prepsum:
                    ftr = prepool.tile([128, KO_F, NB], BF16)
                    wfT = prepool.tile([128, KO_F, A], BF16)
                    wie = prepool.tile([128, KO_E, 4 * D], BF16)
                    embA = prepool.tile([128, KO_E, NROW], BF16)
                    nc.sync.dma_start(out=ftr[:], in_=i_ftr[:])
                    nc.sync.dma_start(out=wfT[:], in_=i_wfT[:])
                    nc.sync.dma_start(out=wie[:], in_=i_wie[:])
                    nc.sync.dma_start(out=embA[:], in_=i_emb[:])
                    for ao in range(A // 128):
                        for c in range(3):
                            ps = prepsum.tile([128, NB // 3], F32, tag="pp")
                            for fo in range(KO_F):
                                nc.tensor.matmul(
                                    ps[:],
                                    wfT[:, fo, ao * 128:(ao + 1) * 128],
                                    ftr[:, fo, c * 384:(c + 1) * 384],
                                    start=(fo == 0), stop=(fo == KO_F - 1),
                                )
                            # add bias (bf+bd) and cast to bf16
                            nc.scalar.activation(
                                imgT[:, ao, c * 384:(c + 1) * 384], ps[:],
                                mybir.ActivationFunctionType.Identity,
                                bias=iab[:, ao:ao + 1],
                            )
                    # eg[(t,j), g] = emb_tj @ Wih_e^T
                    for mch in range(NROW // 128 + 1):
                        m0 = mch * 128
                        msz = min(128, NROW - m0)
                        if msz <= 0:
                            break
                        for c in range(8):
                            ps2 = prepsum.tile([128, 512], F32, tag="pp")
                            for eo in range(KO_E):
                                nc.tensor.matmul(
                                    ps2[:msz, :],
                                    embA[:, eo, m0:m0 + msz],
                                    wie[:, eo, c * 512:(c + 1) * 512],
                                    start=(eo == 0), stop=(eo == KO_E - 1))
                            egs = preo.tile([128, 512], BF16, tag="ego")
                            nc.vector.tensor_copy(out=egs[:msz, :],
                                                  in_=ps2[:msz, :])
                            nc.sync.dma_start(
                                out=d_eg[m0:m0 + msz, c * 512:(c + 1) * 512],
                                in_=egs[:msz, :])

                # --- loop-resident weights ---
                with tc.tile_pool(name="wres", bufs=1) as wrpool:
                    wres = wrpool.tile([128, KO_H, 4 * D], BF16)
                    nc.sync.dma_start(out=wres[:], in_=i_whh[:])

                    KO2 = KO_F + KO_H      # 24 in-loop contraction tiles
                    with tc.tile_pool(name="state", bufs=1) as spool, \
                         tc.tile_pool(name="wt", bufs=2) as wtpool, \
                         tc.tile_pool(name="egp", bufs=2) as egp, \
                         tc.tile_pool(name="work2", bufs=1) as wk2, \
                         tc.tile_pool(name="ps", bufs=8, space="PSUM") as psp:
                        xT = spool.tile([128, KO2, BL], BF16)
                        cst = spool.tile([BL, D], F32)
                        nc.vector.memset(xT[:, KO_F:, :], 0)
                        nc.vector.memset(cst[:], 0)

                        for t in range(T):
                            # ---- h_att = h @ Wd^T  [BL, A] ----
                            ha = psp.tile([BL, A], F32, tag="ps")
                            for ko in range(KO_H):
                                nc.tensor.matmul(
                                    ha[:], xT[:, KO_F + ko, :],
                                    wdT[:, ko, :],
                                    start=(ko == 0), stop=(ko == KO_H - 1),
                                )
                            haS = wk2.tile([BL, A], BF16, tag="haS")
                            nc.vector.tensor_copy(out=haS[:], in_=ha[:])
                            # transpose to a-major [128, ao, BL]
                            hat = wk2.tile([128, A // 128, BL], BF16, tag="hat")
                            for ao in range(A // 128):
                                pt = psp.tile([128, BL], BF16, tag="ps")
                                nc.tensor.transpose(
                                    pt[:], haS[:, ao * 128:(ao + 1) * 128], idb[:])
                                nc.vector.tensor_copy(out=hat[:, ao, :], in_=pt[:])

                            # ---- att1 = relu(img + h_att), scores ----
                            ats = []
                            for ao in range(A // 128):
                                at = wk2.tile([128, NB], BF16, tag=f"at{ao}")
                                atv = at[:].rearrange("p (n j) -> p n j", j=BL)
                                nc.vector.tensor_tensor(
                                    out=atv,
                                    in0=imgT[:, ao, :].rearrange(
                                        "p (n j) -> p n j", j=BL),
                                    in1=hat[:, ao:ao + 1, :].broadcast_to(
                                        [128, N, BL]),
                                    op=mybir.AluOpType.add,
                                )
                                nc.vector.tensor_relu(out=at[:], in_=at[:])
                                ats.append(at)
                            # column-at-a-time so each PSUM accumulation group
                            # is contiguous (start=True clears whole-bank bits)
                            scT = psp.tile([128, N // 4], F32, tag="ps")
                            for ci in range(N // 4):
                                for ao in range(A // 128):
                                    nc.tensor.matmul(
                                        scT[:, ci:ci + 1],
                                        ats[ao][:, ci * 128:(ci + 1) * 128],
                                        waT[:, ao, :],
                                        start=(ao == 0), stop=(ao == A // 128 - 1),
                                    )
                            # scT[(nh,j), ci] = scores[j, 4*ci+nh]
                            esc = wk2.tile([128, N // 4], F32, tag="esc")
                            nc.scalar.activation(
                                esc[:], scT[:], mybir.ActivationFunctionType.Exp)
                            # row sums: rs[j, ci] = sum_nh esc[(nh,j), ci]
                            rs = psp.tile([BL, N // 4], F32, tag="ps")
                            nc.tensor.matmul(rs[:], i32f[:], esc[:],
                                             start=True, stop=True)
                            sums = wk2.tile([BL, 1], F32, tag="sums")
                            nc.vector.tensor_reduce(
                                out=sums[:], in_=rs[:],
                                axis=mybir.AxisListType.X, op=mybir.AluOpType.add)
                            rcp = wk2.tile([BL, 1], F32, tag="rcp")
                            nc.vector.reciprocal(out=rcp[:], in_=sums[:])

                            # ---- diag(e) blocks, aw ----
                            dg = wk2.tile([128, N // 4, BL], BF16, tag="dg")
                            for no in range(N // 4):
                                nc.vector.tensor_scalar(
                                    out=dg[:, no, :], in0=i32b[:],
                                    scalar1=esc[:, no:no + 1], scalar2=None,
                                    op0=mybir.AluOpType.mult)
                            awb = wk2.tile([BL, FT], BF16, tag="awb", bufs=2)
                            for c in range(4):
                                pw = psp.tile([BL, 512], F32, tag="ps")
                                for no in range(N // 4):
                                    nc.tensor.matmul(
                                        pw[:], dg[:, no, :],
                                        fnb[:, no, c * 512:(c + 1) * 512],
                                        start=(no == 0), stop=(no == N // 4 - 1))
                                # normalize by softmax denom while evacuating
                                nc.vector.tensor_scalar(
                                    out=awb[:, c * 512:(c + 1) * 512], in0=pw[:],
                                    scalar1=rcp[:], scalar2=None,
                                    op0=mybir.AluOpType.mult)
                            nc.sync.dma_start(out=o_aw[t], in_=awb[:])

                            # ---- x^T assembly ----
                            for fo in range(KO_F):
                                pt = psp.tile([128, BL], BF16, tag="ps")
                                nc.tensor.transpose(
                                    pt[:], awb[:, fo * 128:(fo + 1) * 128], idb[:])
                                nc.vector.tensor_copy(out=xT[:, fo, :], in_=pt[:])

                            # emb-gates contribution for this step
                            egs = egp.tile([BL, 4 * D], BF16, tag="egs")
                            nc.sync.dma_start(
                                out=egs[:], in_=d_eg[t * BL:(t + 1) * BL, :])

                            # ---- gates = x @ W^T  [BL, 4D] ----
                            gps = [psp.tile([BL, 512], F32, tag="ps",
                                            name=f"g{t}_{c}")
                                   for c in range(8)]
                            for ko in range(KO2):
                                if ko < KO_F:
                                    wt = wtpool.tile([128, 4 * D], BF16, tag="wt")
                                    nc.sync.dma_start(out=wt[:], in_=i_wstr[ko])
                                for c in range(8):
                                    sl = slice(c * 512, (c + 1) * 512)
                                    rhs = wt[:, sl] if ko < KO_F \
                                        else wres[:, ko - KO_F, sl]
                                    nc.tensor.matmul(
                                        gps[c][:], xT[:, ko, :], rhs,
                                        start=(ko == 0), stop=(ko == KO2 - 1))

                            # ---- LSTM elementwise (b-major [BL, D]) ----
                            si = wk2.tile([BL, D], BF16, tag="si")
                            sf = wk2.tile([BL, D], BF16, tag="sf")
                            tg = wk2.tile([BL, D], BF16, tag="tg")
                            so = wk2.tile([BL, D], BF16, tag="so")
                            acts = [(si, 0, mybir.ActivationFunctionType.Sigmoid),
                                    (sf, 2, mybir.ActivationFunctionType.Sigmoid),
                                    (tg, 4, mybir.ActivationFunctionType.Tanh),
                                    (so, 6, mybir.ActivationFunctionType.Sigmoid)]
                            for h_ in range(2):
                                s = slice(h_ * 512, (h_ + 1) * 512)
                                for dst, base, fn in acts:
                                    tmp = wk2.tile([BL, 512], F32, tag="gtmp",
                                                   bufs=3, name=f"tm{t}{h_}{base}")
                                    nc.vector.tensor_tensor(
                                        out=tmp[:], in0=gps[base + h_][:],
                                        in1=egs[:, (base + h_) * 512:
                                                (base + h_ + 1) * 512],
                                        op=mybir.AluOpType.add)
                                    nc.scalar.activation(dst[:, s], tmp[:], fn)
                            tt = wk2.tile([BL, D], F32, tag="tt")
                            nc.vector.tensor_tensor(out=tt[:], in0=si[:], in1=tg[:],
                                                    op=mybir.AluOpType.mult)
                            nc.vector.tensor_tensor(out=cst[:], in0=cst[:], in1=sf[:],
                                                    op=mybir.AluOpType.mult)
                            nc.vector.tensor_tensor(out=cst[:], in0=cst[:], in1=tt[:],
                                                    op=mybir.AluOpType.add)
                            tch = wk2.tile([BL, D], F32, tag="tch")
                            nc.scalar.activation(tch[:], cst[:],
                                mybir.ActivationFunctionType.Tanh)
                            hb = wk2.tile([BL, D], BF16, tag="hb", bufs=2)
                            nc.vector.tensor_tensor(out=hb[:], in0=so[:], in1=tch[:],
                                                    op=mybir.AluOpType.mult)
                            nc.sync.dma_start(out=o_h[t], in_=hb[:])
                            # h^T into xT
                            for do in range(KO_H):
                                pt = psp.tile([128, BL], BF16, tag="ps")
                                nc.tensor.transpose(
                                    pt[:], hb[:, do * 128:(do + 1) * 128], idb[:])
                                nc.vector.tensor_copy(
                                    out=xT[:, KO_F + do, :], in_=pt[:])
    nc.compile()
    return nc


# ----------------------------------------------------------------------------
# NEFF-B: deferred vocab projection
# ----------------------------------------------------------------------------

def build_neff_b(counts):
    KOB = KO_H + KO_F    # 24 contraction tiles (D then FT)
    nc = bacc.Bacc("TRN2")
    i_whT = nc.dram_tensor("whT", [128, KO_H, VLP], BF16, kind="ExternalInput")
    i_wiT = nc.dram_tensor("wiT", [128, KO_F, VLP], BF16, kind="ExternalInput")
    i_xbT = nc.dram_tensor("xbT", [T, 128, KOB, B], BF16, kind="ExternalInput")
    o_p = nc.dram_tensor("po", [VLP // 128, 128, T, B], F32, kind="ExternalOutput")

    with TileContext(nc) as tc:
        with tc.tile_pool(name="w", bufs=1) as wpool, \
             tc.tile_pool(name="x", bufs=2) as xpool, \
             tc.tile_pool(name="o", bufs=4) as opool, \
             tc.tile_pool(name="ps", bufs=4, space="PSUM") as psp:
            whT = wpool.tile([128, KO_H, VLP], BF16)
            wiT = wpool.tile([128, KO_F, VLP], BF16)
            nc.sync.dma_start(out=whT[:], in_=i_whT[:])
            nc.sync.dma_start(out=wiT[:], in_=i_wiT[:])
            for t in range(T):
                nt = int(counts[t])
                if nt <= 0:
                    continue
                xb = xpool.tile([128, KOB, B], BF16, tag="xb")
                nc.sync.dma_start(out=xb[:, :, :nt], in_=i_xbT[t, :, :, :nt])
                for mo in range(VLP // 128):
                    pp = psp.tile([128, B], F32, tag="pp")
                    for ko in range(KOB):
                        w = whT[:, ko, mo * 128:(mo + 1) * 128] if ko < KO_H \
                            else wiT[:, ko - KO_H, mo * 128:(mo + 1) * 128]
                        nc.tensor.matmul(pp[:, :nt], w, xb[:, ko, :nt],
                                         start=(ko == 0), stop=(ko == KOB - 1))
                    ob = opool.tile([128, B], F32, tag="ob")
                    nc.vector.tensor_copy(out=ob[:, :nt], in_=pp[:, :nt])
                    nc.sync.dma_start(out=o_p[mo, :, t, :nt], in_=ob[:, :nt])
    nc.compile()
    return nc


# ----------------------------------------------------------------------------
# Host orchestration
# ----------------------------------------------------------------------------

def _prep_a_inputs(feats_s, emb_x, Wf, bf, Wd, bd, Wa, Wih, Whh):
    """Per-core input dicts for NEFF-A."""
    WihT = np.ascontiguousarray(Wih.T)       # [FT+E, 4D]
    WhhT = np.ascontiguousarray(Whh.T)       # [D, 4D]
    wstr = WihT[:FT].reshape(KO_F, 128, 4 * D).astype(NPBF)
    whh = np.ascontiguousarray(
        WhhT.reshape(KO_H, 128, 4 * D).transpose(1, 0, 2)).astype(NPBF)
    wie = np.ascontiguousarray(
        WihT[FT:].reshape(KO_E, 128, 4 * D).transpose(1, 0, 2)).astype(NPBF)
    wfT = np.ascontiguousarray(Wf.T.reshape(KO_F, 128, A).transpose(1, 0, 2)).astype(NPBF)
    wdT = np.ascontiguousarray(Wd.T.reshape(KO_H, 128, A).transpose(1, 0, 2)).astype(NPBF)
    waT = np.ascontiguousarray(Wa[0].reshape(A // 128, 128, 1).transpose(1, 0, 2)).astype(NPBF)
    iab = np.ascontiguousarray((bf + bd).reshape(A // 128, 128).T).astype(np.float32)
    i32 = np.tile(_ident(BL, np.float32), (4, 1))           # [128, 32]
    idb = _ident(32, NPBF)
    in_maps = []
    for k in range(NC):
        fl = feats_s[k::NC]                                  # [32, 36, FT]
        fnb = np.ascontiguousarray(
            fl.reshape(BL, N // 4, 4, FT).transpose(2, 0, 1, 3)
        ).reshape(128, N // 4, FT).astype(NPBF)
        # fl.T: [FT, N, BL] -> [fo, p, n, j] -> [p, fo, n*32+j]
        ftr = np.ascontiguousarray(
            fl.transpose(2, 1, 0).reshape(KO_F, 128, N * BL).transpose(1, 0, 2)
        ).astype(NPBF)
        el = emb_x[k::NC]                                    # [32, T, E]
        embA = np.ascontiguousarray(
            el.transpose(2, 1, 0).reshape(KO_E, 128, T, BL)
            .transpose(1, 0, 2, 3).reshape(128, KO_E, T * BL)
        ).astype(NPBF)
        in_maps.append(dict(
            fnb=fnb, ftr=ftr, wfT=wfT, wdT=wdT, waT=waT, iab=iab,
            whh=whh, wie=wie, wstr=wstr, embA=embA,
            i32b=i32.astype(NPBF), i32f=i32, idb=idb,
        ))
    return in_maps


def _prep_b_inputs(h_all, aw_all, Wh, Wi):
    """h_all [B, T, D], aw_all [B, T, FT] (sorted order, bf16-valued f32)."""
    xbT = np.concatenate(
        [h_all.transpose(1, 2, 0).reshape(T, KO_H, 128, B),
         aw_all.transpose(1, 2, 0).reshape(T, KO_F, 128, B)], axis=1)
    xbT = np.ascontiguousarray(xbT.transpose(0, 2, 1, 3)).astype(NPBF)
    in_maps = []
    for k in range(NC):
        wh = np.zeros((D, VLP), np.float32)
        wi = np.zeros((FT, VLP), np.float32)
        wh[:, :VL] = Wh[k * VL:(k + 1) * VL].T
        wi[:, :VL] = Wi[k * VL:(k + 1) * VL].T
        whT = np.ascontiguousarray(
            wh.reshape(KO_H, 128, VLP)).transpose(1, 0, 2)
        wiT = np.ascontiguousarray(
            wi.reshape(KO_F, 128, VLP)).transpose(1, 0, 2)
        in_maps.append(dict(
            whT=np.ascontiguousarray(whT).astype(NPBF),
            wiT=np.ascontiguousarray(wiT).astype(NPBF),
            xbT=xbT,
        ))
    return in_maps


def kernel(feats, sequences, sizes, emb, Wf, bf, Wd, bd, Wa, ba,
           Wih, Whh, bih, bhh, Wh, bh, Wi, bi):
    global LAST_EXEC_NS
    LAST_EXEC_NS = []
    feats = np.asarray(feats, np.float32)
    sizes_np = np.asarray(sizes)
    seq_np = np.asarray(sequences)
    order = np.argsort(-sizes_np[:, 0], kind="stable")
    dec_len = (sizes_np[order, 0] - 1).astype(np.int64)
    counts = np.array([(dec_len > t).sum() for t in range(T)], np.int64)
    feats_s = feats[order]
    emb_x = np.asarray(emb, np.float32)[seq_np[order][:, :T]]   # [B, T, E]

    assert not (np.any(bih) or np.any(bhh) or np.any(ba)), \
        "nonzero lstm/att scalar biases not folded in this build"

    if "a" not in _cache:
        _cache["a"] = build_neff_a()
    nc_a = _cache["a"]
    in_a = _prep_a_inputs(feats_s, emb_x, np.asarray(Wf), np.asarray(bf),
                          np.asarray(Wd), np.asarray(bd), np.asarray(Wa),
                          np.asarray(Wih), np.asarray(Whh))
    res_a = bass_utils.run_bass_kernel_spmd(
        nc_a, in_a, core_ids=list(range(NC)), trace=TRACE)
    if res_a.exec_time_ns:
        LAST_EXEC_NS.append(res_a.exec_time_ns)

    # reassemble h_all/aw_all in sorted order
    h_all = np.zeros((B, T, D), np.float32)
    aw_all = np.zeros((B, T, FT), np.float32)
    for k in range(NC):
        h_all[k::NC] = res_a.results[k]["ho"].astype(np.float32).transpose(1, 0, 2)
        aw_all[k::NC] = res_a.results[k]["awo"].astype(np.float32).transpose(1, 0, 2)

    key = ("b", tuple(counts.tolist()))
    if key not in _cache:
        _cache[key] = build_neff_b(counts)
    nc_b = _cache[key]
    in_b = _prep_b_inputs(h_all, aw_all, np.asarray(Wh), np.asarray(Wi))
    res_b = bass_utils.run_bass_kernel_spmd(
        nc_b, in_b, core_ids=list(range(NC)), trace=TRACE)
    if res_b.exec_time_ns:
        LAST_EXEC_NS.append(res_b.exec_time_ns)

    out = np.zeros((B, T, V), np.float32)
    bias_v = np.asarray(bh, np.float32) + 0.0
    bias_i = np.asarray(bi, np.float32)
    for k in range(NC):
        po = res_b.results[k]["po"].reshape(VLP, T, B)[:VL]   # [VL, T, B]
        for t in range(T):
            nt = int(counts[t])
            if nt:
                out[:nt, t, k * VL:(k + 1) * VL] = po[:, t, :nt].T
    if np.any(bias_v) or np.any(bias_i):
        bb = (bias_v + bias_i)[None, None, :]
        for t in range(T):
            nt = int(counts[t])
            out[:nt, t, :] += bb[0]
    return out
